# revision 5
# baseline (speedup 1.0000x reference)
"""GNN message-passing (2-layer GraphConv + fetch/linear) on 8 TRN2 NeuronCores.

Strategy (self-contained; shapes hardcoded for the target problem):
  - Nodes dst-sharded across 8 cores (25000/core). Each core owns the edges
    whose dst falls in its shard; the full feature table is replicated so the
    per-edge gather h[src] is core-local (sharding hint: edges colocated with
    dst, replicate small weights, exchange src features between layers).
  - Per core the edge list is reordered into cells (owner-window o, 512-dst
    superblock sb) padded to a fixed (compile-time, max-across-cores) multiple
    of 128 so all 8 cores run one identical SPMD program.
  - dma_gather (int16 indices, one 512B/256B row per edge) pulls src rows into
    SBUF; a per-chunk indicator built with one DVE tensor_scalar
    (iota==dstslot)*norm_src feeds TensorE matmuls that accumulate the
    segment-sum in PSUM: aggT[f, slot] += X_chunk.T @ ind.
  - Per 128-dst block: R = aggT_b.T @ W in PSUM, then one ScalarE activation
    relu(R * scale) writes the next-layer table row block (scale folds the
    dst-side norm; for layer 1 the table value is h1*norm so scale=norm^2).
  - One AllGather (ncfw collective) replicates h1*norm to all cores between
    the layers. Final stage: dma_gather of the fetched rows, transpose,
    matmul with w3^T, bias.
"""

import os
import sys

sys.path.insert(0, "/opt/trn_rl_repo")

import numpy as np

import concourse.bacc as bacc
import concourse.bass as bass
import concourse.mybir as mybir
import concourse.tile as tile
from concourse.bass_utils import run_bass_kernel_spmd
from concourse.library_config import mlp as mlp_lib

# ---------------------------------------------------------------- config

N_NODES = 200000
N_EDGES = 3200000
NUM_GRAPHS = 100
NODES_PER_GRAPH = 2000
D = 128
D_OUT = 64
NC = 8                       # cores
S = N_NODES // NC            # 25000 dst nodes per core
SB = 512                     # dst slots per superblock (one PSUM bank)
NSB = (S + SB - 1) // SB     # 49
S_PAD = NSB * SB             # 25088
G_SB = 4                     # superblocks per gather group
NG = (NSB + G_SB - 1) // G_SB
NBLK = S_PAD // 128          # 196 dst blocks per core
MAX_CALL = 1024              # dma_gather ring limit (descs)
N_QUEUES = 4

TBL_DT = mybir.dt.float16    # table / gather / indicator dtype
TBL_NP = np.float16
ACC_DT = mybir.dt.float32

last_result = None           # BassKernelResults of the most recent run

# ---------------------------------------------------------------- host plan


def _roundup(x, m):
    return (x + m - 1) // m * m


class Plan:
    """Host-side reorganization of the edge list into the SPMD structure."""

    def __init__(self, src, dst):
        src = np.asarray(src).astype(np.int64)
        dst = np.asarray(dst).astype(np.int64)
        deg = np.bincount(dst, minlength=N_NODES).astype(np.float64)
        self.norm = (1.0 / np.sqrt(np.clip(deg, 1.0, None))).astype(np.float32)

        owner = src // S
        core = dst // S
        dloc = dst - core * S
        sb = dloc // SB

        # per-core per-cell counts; cell id = o * NSB + sb
        ncell = NC * NSB
        cellid = owner * NSB + sb
        counts = np.zeros((NC, ncell), np.int64)
        for c in range(NC):
            m = core == c
            counts[c] = np.bincount(cellid[m], minlength=ncell)
        target = _roundup(counts.max(axis=0), 128)  # [ncell]

        # cell order in the padded stream: for g, for o, for sb in g
        order = []
        for g in range(NG):
            sbs = range(g * G_SB, min((g + 1) * G_SB, NSB))
            for o in range(NC):
                for s_ in sbs:
                    order.append(o * NSB + s_)
        order = np.array(order, np.int64)
        base = np.zeros(ncell, np.int64)
        base[order] = np.concatenate([[0], np.cumsum(target[order])[:-1]])
        self.E_pad = int(target.sum())
        self.E_pad = _roundup(self.E_pad, 128)
        self.target = target
        self.cell_base = base
        self.cell_order = order

        # gather calls: per (g, o) contiguous run split into <=MAX_CALL pieces
        self.calls = []  # (g, o, start, n)
        self.g_bounds = []  # (g, edge_start, edge_end)
        for g in range(NG):
            sbs = list(range(g * G_SB, min((g + 1) * G_SB, NSB)))
            g_start = None
            for o in range(NC):
                cells = [o * NSB + s_ for s_ in sbs]
                run = int(sum(target[ci] for ci in cells))
                if run == 0:
                    continue
                start = int(base[cells[0]])
                if g_start is None:
                    g_start = start
                pos = start
                left = run
                while left > 0:
                    n = min(MAX_CALL, left)
                    self.calls.append((g, o, pos, n))
                    pos += n
                    left -= n
            self.g_bounds.append((g, g_start, pos))

        # chunk map: for each 128-slot chunk: (g, sb, first-of-sb, last-of-sb)
        nchunk = self.E_pad // 128
        chunk_sb = np.full(nchunk, -1, np.int64)
        for ci in order:
            if target[ci] == 0:
                continue
            o, s_ = divmod(ci, NSB)
            c0 = base[ci] // 128
            c1 = (base[ci] + target[ci]) // 128
            chunk_sb[c0:c1] = s_
        self.chunk_sb = chunk_sb
        first = {}
        last = {}
        for t in range(nchunk):
            s_ = chunk_sb[t]
            if s_ < 0:
                continue
            if s_ not in first:
                first[s_] = t
            last[s_] = t
        self.sb_first_chunk = first
        self.sb_last_chunk = last

        # per-core padded edge arrays
        self.idx16 = np.zeros((NC, self.E_pad), np.int16)
        self.slot = np.full((NC, self.E_pad), -1.0, np.float32)
        self.nsrc = np.zeros((NC, self.E_pad), np.float32)
        for c in range(NC):
            m = core == c
            cid = cellid[m]
            srt = np.argsort(cid, kind="stable")
            cid_s = cid[srt]
            # rank within cell
            cnt = counts[c]
            cell_starts = np.zeros(ncell + 1, np.int64)
            cell_starts[1:] = np.cumsum(cnt)
            ranks = np.arange(cid_s.size) - cell_starts[cid_s]
            pos = base[cid_s] + ranks
            es = src[m][srt]
            ed_loc = dloc[m][srt]
            self.idx16[c, pos] = (es - (es // S) * S).astype(np.int16)
            self.slot[c, pos] = (ed_loc % SB).astype(np.float32)
            self.nsrc[c, pos] = self.norm[es]

    def wrap_idx(self, c):
        a = self.idx16[c].reshape(-1, 16).T  # [16, E/16]
        return np.tile(a, (8, 1)).copy()

    def chunk_cols(self, arr_c, dtype):
        return arr_c.reshape(-1, 128).T.astype(dtype).copy()  # [128, E/128]


# ---------------------------------------------------------------- bass build


def _emit_layer(nc, tc, plan, pools, consts, table, win_rows, bounce, w_tile,
                scale_np, use_nsrc, slabs, qoff=0):
    """Emit one GraphConv layer. table: DRAM tensor gathered per owner-window
    of win_rows rows; bounce: DRAM [S_PAD, D] written with the layer output
    (already scaled for the next layer's use)."""
    E_pad = plan.E_pad
    iota_t = consts["iota"]
    scale_t = consts[scale_np]
    gp, indp, aggp, aggsb, rp, stp = (pools[k] for k in
                                      ("g", "ind", "agg", "aggsb", "r", "st"))
    idx_sl, slot_sl, nsrc_sl = slabs

    call_by_g = {}
    for (g, o, start, n) in plan.calls:
        call_by_g.setdefault(g, []).append((o, start, n))

    qn = qoff
    for g, g_start, g_end in plan.g_bounds:
        # slab loads for this group
        i0, i1 = g_start // 16, g_end // 16
        c0, c1 = g_start // 128, g_end // 128
        idx_t = idx_sl["pool"].tile([128, i1 - i0], mybir.dt.int16, tag="idx")
        nc.sync.dma_start(idx_t[:], idx_sl["dram"][:, i0:i1])
        slot_t = slot_sl["pool"].tile([128, c1 - c0], mybir.dt.float32, tag="slot")
        nc.sync.dma_start(slot_t[:], slot_sl["dram"][:, c0:c1])
        if use_nsrc:
            nsrc_t = nsrc_sl["pool"].tile([128, c1 - c0], mybir.dt.float32, tag="nsrc")
            nc.sync.dma_start(nsrc_t[:], nsrc_sl["dram"][:, c0:c1])

        # gather calls for this group, round-robin across SWDGE queues
        tiles = []  # (edge_start, n, tile)
        for (o, start, n) in call_by_g[g]:
            gt = gp.tile([128, MAX_CALL // 128, D], TBL_DT, tag="g")
            nc.gpsimd.dma_gather(
                gt[:, : n // 128, :],
                table[o * win_rows: o * win_rows + win_rows, :],
                idx_t[:, (start - g_start) // 16: (start - g_start + n) // 16],
                n, n, D, queue_num=qn % N_QUEUES)
            qn += 1
            tiles.append((start, n, gt))

        # per-sb PSUM accumulators for this group
        agg_tiles = {}
        for (start, n, gt) in tiles:
            for k in range(n // 128):
                t = (start + k * 128) // 128  # global chunk id
                s_ = plan.chunk_sb[t]
                if s_ < 0:
                    continue
                if s_ not in agg_tiles:
                    agg_tiles[s_] = aggp.tile([128, SB], ACC_DT, tag="agg",
                                              name=f"agg_sb{s_}")
                ind = indp.tile([128, SB], TBL_DT, tag="ind")
                col = t - c0
                if use_nsrc:
                    nc.vector.tensor_scalar(
                        ind[:], iota_t[:], slot_t[:, col:col + 1],
                        nsrc_t[:, col:col + 1],
                        mybir.AluOpType.is_equal, mybir.AluOpType.mult)
                else:
                    nc.vector.tensor_scalar(
                        ind[:], iota_t[:], slot_t[:, col:col + 1], None,
                        mybir.AluOpType.is_equal)
                nc.tensor.matmul(
                    agg_tiles[s_][:], lhsT=gt[:, k, :], rhs=ind[:],
                    start=(t == plan.sb_first_chunk[s_]),
                    stop=(t == plan.sb_last_chunk[s_]))

        # drain completed superblocks
        for s_ in sorted(agg_tiles):
            aggT = aggsb.tile([128, SB], TBL_DT, tag="aggsb")
            nc.scalar.activation(aggT[:], agg_tiles[s_][:],
                                 mybir.ActivationFunctionType.Copy)
            stage = stp.tile([128, SB // 128, D], TBL_DT, tag="st")
            for b in range(SB // 128):
                blk = s_ * (SB // 128) + b
                r = rp.tile([128, D], ACC_DT, tag="r")
                nc.tensor.matmul(r[:], lhsT=aggT[:, b * 128:(b + 1) * 128],
                                 rhs=w_tile[:], start=True, stop=True)
                nc.scalar.activation(stage[:, b, :], r[:],
                                     mybir.ActivationFunctionType.Relu,
                                     scale=scale_t[:, blk:blk + 1])
            nc.sync.dma_start(
                bounce[s_ * SB:(s_ + 1) * SB, :].rearrange(
                    "(c p) f -> p c f", p=128),
                stage[:])
    return qn


def build_bass(plan, fetch_plan, weights):
    nc = bacc.Bacc("TRN2", target_bir_lowering=False,
                   num_swdge_queues=N_QUEUES)
    w1, b1, w2, b2, w3, b3 = weights
    assert abs(b1).max() == 0 and abs(b2).max() == 0, \
        "nonzero conv bias not supported by this build"

    E_pad = plan.E_pad
    table0 = nc.dram_tensor("table0", [N_NODES, D], TBL_DT,
                            kind="ExternalInput")
    idx_d = nc.dram_tensor("idx", [128, E_pad // 16], mybir.dt.int16,
                           kind="ExternalInput")
    slot_d = nc.dram_tensor("slot", [128, E_pad // 128], mybir.dt.float32,
                            kind="ExternalInput")
    nsrc_d = nc.dram_tensor("nsrc", [128, E_pad // 128], mybir.dt.float32,
                            kind="ExternalInput")
    iota_d = nc.dram_tensor("iota", [128, SB], TBL_DT, kind="ExternalInput")
    norm_d = nc.dram_tensor("normsc", [128, NBLK], mybir.dt.float32,
                            kind="ExternalInput")
    norm2_d = nc.dram_tensor("normsc2", [128, NBLK], mybir.dt.float32,
                             kind="ExternalInput")
    w1_d = nc.dram_tensor("w1t", [D, D], TBL_DT, kind="ExternalInput")
    w2_d = nc.dram_tensor("w2t", [D, D], TBL_DT, kind="ExternalInput")
    w3_d = nc.dram_tensor("w3t", [D, D_OUT], TBL_DT, kind="ExternalInput")
    b3_d = nc.dram_tensor("b3c", [D_OUT, 1], mybir.dt.float32,
                          kind="ExternalInput")
    ident_d = nc.dram_tensor("ident", [128, 128], TBL_DT,
                             kind="ExternalInput")
    fidx_d = nc.dram_tensor("fidx", [128, 8], mybir.dt.int16,
                            kind="ExternalInput")
    fcnt_d = nc.dram_tensor("fcnt", [1, 1], mybir.dt.uint32,
                            kind="ExternalInput")
    y_d = nc.dram_tensor("y", [D_OUT, 128], mybir.dt.float32,
                         kind="ExternalOutput")

    h1_bounce = nc.dram_tensor("h1b", [S_PAD, D], TBL_DT)
    table1 = nc.dram_tensor("table1", [NC * S_PAD, D], TBL_DT,
                            addr_space="Shared")
    h2_loc = nc.dram_tensor("h2loc", [S_PAD, D], TBL_DT)

    with tile.TileContext(nc) as tc:
        nc.gpsimd.load_library(mlp_lib)
        with (
            tc.tile_pool(name="consts", bufs=1) as cpool,
            tc.tile_pool(name="idx", bufs=2) as idxp,
            tc.tile_pool(name="slot", bufs=2) as slotp,
            tc.tile_pool(name="nsrc", bufs=2) as nsrcp,
            tc.tile_pool(name="g", bufs=8) as gp,
            tc.tile_pool(name="ind", bufs=6) as indp,
            tc.tile_pool(name="aggsb", bufs=3) as aggsbp,
            tc.tile_pool(name="st", bufs=3) as stp,
            tc.tile_pool(name="agg", bufs=5, space="PSUM") as aggp,
            tc.tile_pool(name="r", bufs=2, space="PSUM") as rp,
        ):
            consts = {}
            for nm, dr, shape, dt in (
                ("iota", iota_d, [128, SB], TBL_DT),
                ("norm", norm_d, [128, NBLK], mybir.dt.float32),
                ("norm2", norm2_d, [128, NBLK], mybir.dt.float32),
                ("w1", w1_d, [D, D], TBL_DT),
                ("w2", w2_d, [D, D], TBL_DT),
                ("w3", w3_d, [D, D_OUT], TBL_DT),
                ("b3", b3_d, [D_OUT, 1], mybir.dt.float32),
                ("ident", ident_d, [128, 128], TBL_DT),
                ("fidx", fidx_d, [128, 8], mybir.dt.int16),
            ):
                t = cpool.tile(shape, dt, tag=nm)
                nc.sync.dma_start(t[:], dr[:])
                consts[nm] = t

            pools = {"g": gp, "ind": indp, "agg": aggp, "aggsb": aggsbp,
                     "r": rp, "st": stp}
            slabs = ({"pool": idxp, "dram": idx_d},
                     {"pool": slotp, "dram": slot_d},
                     {"pool": nsrcp, "dram": nsrc_d})

            qn = _emit_layer(nc, tc, plan, pools, consts, table0, S,
                             h1_bounce, consts["w1"], "norm2", True, slabs)

            nc.gpsimd.collective_compute(
                "AllGather", mybir.AluOpType.bypass,
                replica_groups=[list(range(NC))],
                ins=[h1_bounce.ap().opt()],
                outs=[table1.ap().opt()])

            _emit_layer(nc, tc, plan, pools, consts, table1, S_PAD,
                        h2_loc, consts["w2"], "norm", False, slabs, qoff=qn)

            # final fetch + linear
            fcnt_reg = nc.gpsimd.alloc_register("fcnt_reg")
            nc.gpsimd.reg_load(fcnt_reg, fcnt_d[0:1, 0:1])
            fx = gp.tile([128, 1, D], TBL_DT, tag="g")
            nc.vector.memset(fx[:], 0.0)
            nc.gpsimd.dma_gather(fx[:], h2_loc[:], consts["fidx"][:],
                                 128, fcnt_reg, D)
            xt_ps = rp.tile([128, 128], TBL_DT, tag="r")
            nc.tensor.transpose(xt_ps[:], fx[:, 0, :], consts["ident"][:])
            xt = aggsbp.tile([128, 128], TBL_DT, tag="aggsb")
            nc.scalar.activation(xt[:], xt_ps[:],
                                 mybir.ActivationFunctionType.Copy)
            out_ps = rp.tile([D_OUT, 128], ACC_DT, tag="r")
            nc.tensor.matmul(out_ps[:], lhsT=consts["w3"][:], rhs=xt[:],
                             start=True, stop=True)
            out_sb = stp.tile([D_OUT, 128], mybir.dt.float32, tag="st")
            nc.vector.tensor_scalar_add(out_sb[:], out_ps[:],
                                        consts["b3"][:, 0:1])
            nc.sync.dma_start(y_d[:], out_sb[:])
    nc.compile()
    return nc


# ---------------------------------------------------------------- kernel


def kernel(features, src, dst, to_fetch, w1, b1, w2, b2, w3, b3):
    global last_result
    features = np.asarray(features)
    plan = Plan(np.asarray(src), np.asarray(dst))

    # fetch bookkeeping
    gidx = np.asarray(to_fetch).astype(np.int64) + \
        np.arange(NUM_GRAPHS, dtype=np.int64) * NODES_PER_GRAPH
    fown = gidx // S
    floc = gidx - fown * S
    fetch_rows = []   # per core: positions into the 100-row output
    fidx_arr = np.full((NC, 128), -1, np.int16)
    fcnt = np.zeros(NC, np.int64)
    for c in range(NC):
        rows = np.where(fown == c)[0]
        fetch_rows.append(rows)
        fidx_arr[c, : rows.size] = floc[rows].astype(np.int16)
        fcnt[c] = rows.size

    weights = (np.asarray(w1), np.asarray(b1), np.asarray(w2),
               np.asarray(b2), np.asarray(w3), np.asarray(b3))
    nc = build_bass(plan, None, weights)

    # ---- per-core inputs
    tbl0 = features.astype(TBL_NP)
    iota = np.tile(np.arange(SB, dtype=TBL_NP)[None, :], (128, 1))
    ident = np.eye(128, dtype=TBL_NP)
    w1t = weights[0].astype(TBL_NP)          # [in, out] == lhs-free layout
    w2t = weights[2].astype(TBL_NP)
    w3t = weights[4].T.astype(TBL_NP)        # [128, 64]
    b3c = weights[5].reshape(D_OUT, 1).astype(np.float32)

    in_maps = []
    for c in range(NC):
        # dst-side norm per padded local block layout [p, blk]
        r = np.arange(S_PAD)
        vals = np.where(r < S, plan.norm[c * S + np.minimum(r, S - 1)], 1.0)
        nrm = vals.reshape(NBLK, 128).T.astype(np.float32)
        wrap16 = np.zeros((128, 8), np.int16)
        wrap16[:16] = fidx_arr[c].reshape(8, 16).T
        wrap16 = np.tile(wrap16[:16], (8, 1))
        in_maps.append({
            "table0": tbl0,
            "idx": plan.wrap_idx(c),
            "slot": plan.chunk_cols(plan.slot[c], np.float32),
            "nsrc": plan.chunk_cols(plan.nsrc[c], np.float32),
            "iota": iota,
            "normsc": nrm,
            "normsc2": (nrm * nrm).astype(np.float32),
            "w1t": w1t, "w2t": w2t, "w3t": w3t, "b3c": b3c,
            "ident": ident,
            "fidx": wrap16,
            "fcnt": np.array([[fcnt[c]]], np.uint32),
        })

    res = run_bass_kernel_spmd(nc, in_maps, core_ids=list(range(NC)),
                               trace=bool(os.environ.get("BASS_TRACE")))
    last_result = res

    out = np.zeros((NUM_GRAPHS, D_OUT), np.float32)
    for c in range(NC):
        yc = res.results[c]["y"]  # [64, 128]
        rows = fetch_rows[c]
        out[rows] = yc[:, : rows.size].T
    return out
